# revision 2
# baseline (speedup 1.0000x reference)
"""Trainium2 Bass kernel v3 for 2D single-level DWT (coif1, symmetric padding).

Input  x: (4, 64, 512, 512) fp32
Output  : (4, 256, 258, 258) fp32  -- per input channel: [cA, cH, cV, cD]

v9: input transfers issue on the scalar HWDGE ring (4 big transfers,
~0.5us of ACT issue cost each), outputs on the sync ring - the two DMA
streams no longer share one FIFO, which was serializing output
completions behind 12us input transfers (system-wide 4.5us stalls +
HAM re-throttles every other block in the v8 trace). Last block's
outputs store per-image so the drain tail shrinks.
"""

import os
import sys

for _p in ("/opt/trn_rl_repo", "/opt/pypackages"):
    if _p not in sys.path:
        sys.path.append(_p)

os.environ.setdefault("JAX_COMPILATION_CACHE_DIR", "/tmp/jax_comp_cache")
os.environ.setdefault("JAX_PERSISTENT_CACHE_MIN_COMPILE_TIME_SECS", "10")

import numpy as np

import concourse.bass as bass
import concourse.bacc as bacc
import concourse.mybir as mybir
from concourse.bass_utils import run_bass_kernel_spmd
from concourse.tile import TileContext

N_CORES = 8
H = W = 512
OUT = 258  # (512 + 6 - 1) // 2
IMGS = 32  # images per core (4*64/8)
F32 = mybir.dt.float32
F16 = mybir.dt.float16
F16NP = np.float16

# banded windows: input chunk q (rows 128q..128q+127) only produces
# output rows kh in WIN[q]; adjacent windows overlap by 2 (seam columns)
WIN = ((0, 66), (64, 130), (128, 194), (192, 258))
WINW = 66
KHC = (128, 128)  # kh chunks for pass 2 stationary (rows 256-257 on host)

# pywt coif1 decomposition filters (already flipped: correlation form)
DEC_LO = np.array([-0.01565572813546454, -0.0727326195128539, 0.38486484686420286,
                   0.8525720202122554, 0.3378976624578092, -0.0727326195128539])
DEC_HI = np.array([0.0727326195128539, 0.3378976624578092, -0.8525720202122554,
                   0.38486484686420286, 0.0727326195128539, -0.01565572813546454])
FLEN = 6
PAD = 4
LO_F = DEC_LO[::-1]
HI_F = DEC_HI[::-1]


def _build_R(filt: np.ndarray, n: int = W) -> np.ndarray:
    """Banded [258, 512] operator: out[k] = sum_j filt[j] * x[sym(2k + j - PAD)]."""
    out_len = (n + FLEN - 1) // 2

    def sym(i: int) -> int:
        while i < 0 or i >= n:
            if i < 0:
                i = -i - 1
            if i >= n:
                i = 2 * n - 1 - i
        return i

    R = np.zeros((out_len, n), dtype=np.float64)
    for k in range(out_len):
        for j in range(FLEN):
            R[k, sym(2 * k + j - PAD)] += filt[j]
    return R


def _build_weights() -> np.ndarray:
    """w[p, (q*2+f)*66 + j] = R_f[WIN[q][0] + j, 128q + p], fp16 [128, 528]."""
    Rs = [_build_R(LO_F), _build_R(HI_F)]
    w = np.zeros((128, 8 * WINW), dtype=np.float64)
    for q in range(4):
        lo, hi = WIN[q]
        for f in range(2):
            blk = Rs[f][lo:hi, 128 * q:128 * (q + 1)]  # [66, 128]
            outside = np.concatenate(
                [Rs[f][:lo, 128 * q:128 * (q + 1)],
                 Rs[f][hi:, 128 * q:128 * (q + 1)]])
            assert np.all(outside == 0.0), (q, f)
            w[:, (q * 2 + f) * WINW:(q * 2 + f + 1) * WINW] = blk.T
    return w.astype(F16NP)


_WEIGHTS = _build_weights()
_MODULE = None
PS_BUFS = 4
XPOOL_BUFS = 3
YPOOL_BUFS = 3
SPOOL_BUFS = 3


def _build_module() -> bass.Bass:
    nc = bacc.Bacc("TRN2", target_bir_lowering=False, debug=False)
    # x[ob, p, ((j*4 + q)*512 + c)] = img[8*ob + j, 128q + p, c]
    x_in = nc.declare_dram_parameter("x", [IMGS // 8, 128, 8 * 4 * W], F16,
                                     isOutput=False)
    w_in = nc.declare_dram_parameter("w", [128, 8 * WINW], F16, isOutput=False)
    # y[ib, p, (((jj*2+f)*2 + g)*2 + khc)*258 + kw] = O[4ib+jj, f, g, 128khc+p, kw]
    y_out = nc.declare_dram_parameter("y", [IMGS // 4, 128, 4 * 4 * 2 * OUT], F16,
                                      isOutput=True)

    with TileContext(nc) as tc:
        with (
            tc.tile_pool(name="wpool", bufs=1) as wpool,
            tc.tile_pool(name="xpool", bufs=XPOOL_BUFS) as xpool,
            tc.tile_pool(name="ypool", bufs=YPOOL_BUFS) as ypool,
            tc.tile_pool(name="spool", bufs=SPOOL_BUFS) as spool,
            tc.tile_pool(name="psum", bufs=4, space="PSUM") as pspool,
        ):
            Wt = wpool.tile([128, 8 * WINW], F16)
            nc.sync.dma_start(out=Wt[:], in_=w_in[:])
            Wr = Wt[:]

            # Tiny PE op consuming the weight DMA so later matmuls depend
            # on it via PE program order.
            warm = pspool.tile([128, 512], F32, tag="ps", bufs=PS_BUFS,
                               name="warm")
            nc.tensor.matmul(warm[:, 0:WINW], lhsT=Wr[:, 0:128],
                             rhs=Wr[:, 0:WINW], start=True, stop=True)

            def load_x(ob, split=False):
                Xt = xpool.tile([128, 8 * 4 * W], F16, tag="X", name=f"X_{ob}")
                if split:
                    qc = 2 * 4 * W
                    for si in range(4):
                        nc.scalar.dma_start(out=Xt[:, si * qc:(si + 1) * qc],
                                            in_=x_in[ob][:, si * qc:(si + 1) * qc])
                else:
                    nc.scalar.dma_start(out=Xt[:], in_=x_in[ob])
                return Xt

            def evac(dst, src, ev):
                if ev % 2 == 0:
                    nc.scalar.copy(out=dst, in_=src)
                else:
                    nc.vector.tensor_copy(out=dst, in_=src)

            ev = 0
            xq = [load_x(0, split=True), load_x(1)]
            for ib in range(IMGS // 4):
                if ib % 2 == 0:
                    Xcur = xq.pop(0)
                    Xv = Xcur.rearrange("p (t q c) -> p t q c", t=8, q=4)
                elif (ib + 3) // 2 < IMGS // 8:
                    xq.append(load_x((ib + 3) // 2))
                # [128, (jj, f, g, khc, kw)]
                STG = spool.tile([128, 4 * 4 * 2 * OUT], F16, tag="stg",
                                 name="stg")
                STGv = STG.rearrange("p (jj f g khc k) -> p jj f g khc k",
                                     jj=4, f=2, g=2, khc=2)
                Yts = {}
                for half in range(2):
                    for j in (2 * half, 2 * half + 1):
                        # ---- pass 1 ----
                        Yt = ypool.tile([128, 4 * 2 * OUT], F16, tag="Yt",
                                        name=f"Yt_{j}")
                        Ytv = Yt.rearrange("p (q f k) -> p q f k", q=4, f=2)
                        Yts[j] = Ytv
                        for cc in range(4):
                            ps = pspool.tile([128, 1024], F32, tag="ps",
                                             bufs=PS_BUFS, name="ps1")
                            psv = ps.rearrange("p (f k) -> p f k", f=2)
                            for q in range(4):
                                lhsT = Xv[:, (ib % 2) * 4 + j, q,
                                          128 * cc:128 * (cc + 1)]
                                lo, hi = WIN[q]
                                for f in range(2):
                                    nc.tensor.matmul(
                                        psv[:, f, lo:hi],
                                        lhsT=lhsT,
                                        rhs=Wr[:, (q * 2 + f) * WINW:
                                               (q * 2 + f + 1) * WINW],
                                        start=(q == 0),
                                        stop=(q == 3),
                                    )
                            evac(Ytv[:, cc, :, :], psv[:, :, 0:OUT], ev)
                            ev += 1
                    for j in (2 * half, 2 * half + 1):
                        # ---- pass 2 ----
                        Ytv = Yts[j]
                        for khc in range(2):
                            for f in range(2):
                                ps = pspool.tile([128, 1024], F32, tag="ps",
                                                 bufs=PS_BUFS, name="ps2")
                                psv = ps.rearrange("p (g k) -> p g k", g=2)
                                for q in range(4):
                                    lhsT = Ytv[:, q, f,
                                               128 * khc:128 * (khc + 1)]
                                    lo, hi = WIN[q]
                                    for g in range(2):
                                        nc.tensor.matmul(
                                            psv[:, g, lo:hi],
                                            lhsT=lhsT,
                                            rhs=Wr[:, (q * 2 + g) * WINW:
                                                   (q * 2 + g + 1) * WINW],
                                            start=(q == 0),
                                            stop=(q == 3),
                                        )
                                evac(STGv[:, j, f, :, khc, :],
                                     psv[:, :, 0:OUT], ev)
                                ev += 1
                    # per-half output DMA (2 images' columns, contiguous);
                    # last block stores per image for a shorter drain tail
                    hw_ = 4 * 2 * OUT
                    if ib == IMGS // 4 - 1:
                        for jo in (2 * half, 2 * half + 1):
                            nc.sync.dma_start(
                                out=y_out[ib][:, jo * hw_:(jo + 1) * hw_],
                                in_=STG[:, jo * hw_:(jo + 1) * hw_])
                    else:
                        nc.sync.dma_start(
                            out=y_out[ib][:, 2 * half * hw_:(2 * half + 2) * hw_],
                            in_=STG[:, 2 * half * hw_:(2 * half + 2) * hw_])
    nc.finalize()
    return nc


def _get_module() -> bass.Bass:
    global _MODULE
    if _MODULE is None:
        _MODULE = _build_module()
    return _MODULE


def make_in_maps(x: np.ndarray) -> list[dict]:
    x = np.asarray(x, dtype=np.float32)
    B, C, Hx, Wx = x.shape
    assert (Hx, Wx) == (H, W) and B * C == N_CORES * IMGS
    imgs = x.reshape(B * C, H, W).astype(F16NP)
    in_maps = []
    for k in range(N_CORES):
        blk = imgs[k * IMGS:(k + 1) * IMGS]
        # [32, 512, 512] -> [4, 8, 4, 128, 512] -> [4, 128, 8, 4, 512]
        xp = blk.reshape(IMGS // 8, 8, 4, 128, W).transpose(0, 3, 1, 2, 4)
        xp = np.ascontiguousarray(xp).reshape(IMGS // 8, 128, 8 * 4 * W)
        in_maps.append({"x": xp, "w": _WEIGHTS})
    return in_maps


def _host_edge_rows(x: np.ndarray) -> np.ndarray:
    """Rows kh=256,257 for all images/filters, fp32 on host.

    They only depend on input rows 508..511 (symmetric fold), so this is
    ~6 MFLOP of numpy. Returns [n_imgs, g, f, 2, 258]."""
    imgs = x.reshape(-1, H, W)
    n = imgs.shape[0]
    xe = np.pad(imgs[:, 508:512, :].astype(np.float64),
                ((0, 0), (0, 0), (PAD, PAD)), mode="symmetric")
    z = []
    for filt in (LO_F, HI_F):
        acc = np.zeros((n, 4, OUT), np.float64)
        for j in range(FLEN):
            acc += filt[j] * xe[:, :, j::2][:, :, :OUT]
        z.append(acc)  # W-transform of rows 508..511, filter g
    out = np.empty((n, 2, 2, 2, OUT), dtype=np.float32)  # [i, g, f, t, kw]
    for fi, filt in enumerate((LO_F, HI_F)):
        A = _build_R(filt)[256:258, 508:512]  # [2, 4]
        for gi in range(2):
            out[:, gi, fi] = np.einsum("tr,irk->itk", A, z[gi])
    return out


def _unpack_output(res, edge: np.ndarray) -> np.ndarray:
    cores = []
    for k in range(N_CORES):
        y = np.asarray(res[k]["y"]).astype(np.float32).reshape(
            IMGS // 4, 128, 4, 2, 2, 2, OUT)
        o = np.empty((IMGS, 2, 2, OUT, OUT), dtype=np.float32)
        # y[ib, p, jj, f, g, khc, kw] -> o[4ib+jj, g, f, 128khc+p, kw]
        o[:, :, :, :256, :] = (
            y.transpose(0, 2, 4, 3, 5, 1, 6)  # [ib, jj, g, f, khc, p, kw]
            .reshape(IMGS, 2, 2, 256, OUT)
        )
        o[:, :, :, 256:, :] = edge[k * IMGS:(k + 1) * IMGS]
        cores.append(o)
    return np.concatenate(cores, axis=0)  # [256, g, f, 258, 258]


def kernel(**inputs) -> np.ndarray:
    x = np.asarray(inputs["x"], dtype=np.float32)
    B, C, _, _ = x.shape
    nc = _get_module()
    in_maps = make_in_maps(x)
    edge = _host_edge_rows(x)
    res = run_bass_kernel_spmd(nc, in_maps, list(range(N_CORES))).results
    full = _unpack_output(res, edge)  # [256, g, f, 258, 258]
    out = full.reshape(B, C * 4, OUT, OUT)
    return np.ascontiguousarray(out).astype(np.float32)
